# revision 16
# baseline (speedup 1.0000x reference)
"""Trainium2 Bass kernel for nn_CartTensorMix2 (gnn_message_passing).

Strategy (data-parallel over atoms, 8 cores):
  * Host: shard atoms contiguously across 8 cores, transpose activations to
    feature-major layout, build expanded constant weight matrices so the whole
    per-atom tensor-product pipeline becomes: matmuls (PE) + two elementwise
    multiplies (DVE) + matmuls, all feature-major with atoms on the free dim.
  * Device per 512-atom tile:
      h1 = Wg1.T @ xsT ; s = silu(h1+bg1) ; w = Wg2x_aug.T @ [s;1]   (464 rows)
      per pair-group (l1,l2): hUe = EU.T @ x_l1 ; hVe = EV.T @ x_l2 ; P = hUe*hVe
      Q = Cpg.T @ P (464 rows) ; wQ = w * Q ; atom_outT = R.T @ wQ   (9 rows)
      segment-sum via per-128-block one-hot matmul into a 32-graph window
  * Host: scatter-add the per-tile windows into (G,9), apply cart conversion.
"""
import math
import os
import sys
import numpy as np
from fractions import Fraction
from math import factorial

sys.path.insert(0, '/opt/trn_rl_repo')

import concourse.bass as bass
import concourse.tile as tile
from concourse import bacc, mybir

# ----------------------------------------------------------------------------
# problem constants (hardcoded from the task spec)
# ----------------------------------------------------------------------------
N_ATOMS = 150000
N_CORES = 8
G_MAX = 2000
HID = 16
A_TILE = 512           # atoms per tile (matmul free dim)
WIN = 32               # one-hot graph window per tile
INS = [(0, 0, 0), (0, 2, 2), (1, 1, 0), (1, 1, 1), (1, 1, 2),
       (2, 0, 2), (2, 2, 0), (2, 2, 1), (2, 2, 2)]
PAIR_GROUPS = [
    ((0, 0), [0]),
    ((0, 2), [1]),
    ((1, 1), [2, 3, 4]),
    ((2, 0), [5]),
    ((2, 2), [6, 7, 8]),
]
MUL = {0: 128, 1: 64, 2: 32}
DIM = {0: 1, 1: 3, 2: 5}
USE_F32R = True

f32 = mybir.dt.float32
f32r = mybir.dt.float32r
MMDT = f32r if USE_F32R else f32


def _mm_dt(ap):
    return ap


# ----------------------------------------------------------------------------
# e3nn real Wigner 3j (numpy only; mirrors reference.py)
# ----------------------------------------------------------------------------
def _su2_cg(j1, m1, j2, m2, j3, m3):
    if m3 != m1 + m2:
        return 0.0
    vmin = int(max(-j1 + j2 + m3, -j1 + m1, 0))
    vmax = int(min(j2 + j3 + m1, j3 - j1 + j2, j3 + m3))
    f = lambda n: factorial(round(n))
    C = ((2.0 * j3 + 1.0) * Fraction(
        f(j3 + j1 - j2) * f(j3 - j1 + j2) * f(j1 + j2 - j3) * f(j3 + m3) * f(j3 - m3),
        f(j1 + j2 + j3 + 1) * f(j1 - m1) * f(j1 + m1) * f(j2 - m2) * f(j2 + m2))) ** 0.5
    S = 0
    for v in range(vmin, vmax + 1):
        S += (-1) ** int(v + j2 + m2) * Fraction(
            f(j2 + j3 + m1 - v) * f(j1 - m1 + v),
            f(v) * f(j3 - j1 + j2 - v) * f(j3 + m3 - v) * f(v + j1 - j2 - m3))
    return float(C * S)


def _q_real2complex(l):
    q = np.zeros((2 * l + 1, 2 * l + 1), dtype=complex)
    for m in range(-l, 0):
        q[l + m, l + abs(m)] = 1 / 2 ** 0.5
        q[l + m, l - abs(m)] = -1j / 2 ** 0.5
    q[l, l] = 1.0
    for m in range(1, l + 1):
        q[l + m, l + abs(m)] = (-1) ** m / 2 ** 0.5
        q[l + m, l - abs(m)] = 1j * (-1) ** m / 2 ** 0.5
    return (-1j) ** l * q


def _w3j(l1, l2, l3):
    C = np.zeros((2 * l1 + 1, 2 * l2 + 1, 2 * l3 + 1), dtype=complex)
    for m1 in range(-l1, l1 + 1):
        for m2 in range(-l2, l2 + 1):
            for m3 in range(-l3, l3 + 1):
                C[l1 + m1, l2 + m2, l3 + m3] = _su2_cg(l1, m1, l2, m2, l3, m3)
    C = np.einsum('ij,kl,mn,ikn->jlm', _q_real2complex(l1), _q_real2complex(l2),
                  np.conj(_q_real2complex(l3).T), C)
    C = C.real
    return C / np.linalg.norm(C)


# ----------------------------------------------------------------------------
# constant-matrix construction
# ----------------------------------------------------------------------------
def build_constants(inp):
    W3J = {ls: _w3j(*ls) for ls in set(INS)}
    WU = {0: np.asarray(inp['WU0'], np.float64), 1: np.asarray(inp['WU1'], np.float64),
          2: np.asarray(inp['WU2'], np.float64)}
    WV = {0: np.asarray(inp['WV0'], np.float64), 1: np.asarray(inp['WV1'], np.float64),
          2: np.asarray(inp['WV2'], np.float64)}
    WUn = {l: WU[l] / np.sqrt(MUL[l]) for l in (0, 1, 2)}
    WVn = {l: WV[l] / np.sqrt(MUL[l]) for l in (0, 1, 2)}

    EU, EV, Cpg = {}, {}, {}
    qrows = []
    for (l1, l2), ks in PAIR_GROUPS:
        d1, d2 = DIM[l1], DIM[l2]
        nrow = HID * d1 * d2
        eu = np.zeros((MUL[l1] * d1, nrow))
        ev = np.zeros((MUL[l2] * d2, nrow))
        for u in range(HID):
            for i in range(d1):
                for j in range(d2):
                    c = (u * d1 + i) * d2 + j
                    eu[:, c].reshape(MUL[l1], d1)[:, i] = WUn[l1][:, u]
                    ev[:, c].reshape(MUL[l2], d2)[:, j] = WVn[l2][:, u]
        EU[(l1, l2)] = eu.astype(np.float32)
        EV[(l1, l2)] = ev.astype(np.float32)

        ncol = sum(HID * (2 * INS[k][2] + 1) for k in ks)
        C = np.zeros((nrow, ncol))
        col0 = 0
        for k in ks:
            l3 = INS[k][2]
            d3 = 2 * l3 + 1
            coeff = np.sqrt(d3 / HID)
            w3j = W3J[(l1, l2, l3)]
            for u in range(HID):
                for m in range(d3):
                    col = col0 + u * d3 + m
                    for i in range(d1):
                        for j in range(d2):
                            C[(u * d1 + i) * d2 + j, col] = w3j[i, j, m] * coeff
                    qrows.append((k, u, m))
            col0 += HID * d3
        Cpg[(l1, l2)] = C.astype(np.float32)

    Wg2 = np.asarray(inp['Wg2'], np.float64)
    bg2 = np.asarray(inp['bg2'], np.float64)
    # global Q positions for each (pair-group local) row
    qpos = []
    for (pg, ks) in PAIR_GROUPS:
        for i in range(Q_BLOCK_N[pg]):
            qpos.append(q_global_col(pg, i))
    Wg2x = np.zeros((65, NQ_PAD), np.float32)
    for q, (k, u, m) in enumerate(qrows):
        Wg2x[:64, qpos[q]] = Wg2[:, k * HID + u]
        Wg2x[64, qpos[q]] = bg2[k * HID + u]

    Wp = {0: np.asarray(inp['Wp0'], np.float64), 1: np.asarray(inp['Wp1'], np.float64),
          2: np.asarray(inp['Wp2'], np.float64)}
    outs_order = {0: [], 1: [], 2: []}
    for k, (l1, l2, l3) in enumerate(INS):
        outs_order[l3].append(k)
    OFF = {0: 0, 1: 1, 2: 4}
    R = np.zeros((NQ_PAD, 9), np.float32)
    for q, (k, u, m) in enumerate(qrows):
        l3 = INS[k][2]
        idx = outs_order[l3].index(k)
        R[qpos[q], OFF[l3] + m] = Wp[l3][idx, 0] / np.sqrt(len(outs_order[l3]))

    # cart conversion (host-side): out9 = res_sph9 @ M_cart, reshape (3,3)
    Q_COB = np.concatenate([(2 * l + 1) ** 0.5 * _w3j(1, 1, l).transpose(2, 0, 1)
                            for l in (0, 1, 2)], axis=0)  # (9,3,3)
    CART = np.array([2, 0, 1])
    M_cart = Q_COB[:, CART][:, :, CART].reshape(9, 9).astype(np.float32)

    return dict(EU=EU, EV=EV, Cpg=Cpg, Wg2x=Wg2x, R=R, M_cart=M_cart,
                Wg1=np.asarray(inp['Wg1'], np.float32),
                bg1=np.asarray(inp['bg1'], np.float32).reshape(64, 1))


# ----------------------------------------------------------------------------
# device program
# ----------------------------------------------------------------------------
# Placement of pair-group Q blocks into 128-row tiles. Engine SBUF partition
# windows must be one of (0,<=128), (32,<=32), (64,<=64), (96,<=32), so each
# piece below sits on a legal window: (pg, local_lo, width, tile_idx, tile_off)
Q_PIECES = [
    ((1, 1), 0, 128, 0, 0),
    ((2, 2), 0, 128, 1, 0),
    ((0, 2), 0, 80, 2, 0),
    ((1, 1), 128, 16, 2, 96),
    ((2, 0), 0, 80, 3, 0),
    ((2, 2), 128, 16, 3, 96),
    ((0, 0), 0, 16, 4, 0),
]
N_QT = 5
Q_TILE_SIZES = [128, 128, 128, 128, 32]
NQ_PAD = 544
Q_BLOCK_N = {pg: sum(HID * (2 * INS[k][2] + 1) for k in ks)
             for (pg, ks) in PAIR_GROUPS}


def q_global_col(pg, local):
    for (p, lo, w, ti, toff) in Q_PIECES:
        if p == pg and lo <= local < lo + w:
            return ti * 128 + toff + (local - lo)
    raise ValueError((pg, local))


def _ksplits(n):
    """Split n rows into chunks of <=128."""
    out = []
    o = 0
    while o < n:
        c = min(128, n - o)
        out.append((o, c))
        o += c
    return out


def _msplits_for_pg(pg):
    """Pieces of a pair-group's Q columns: (local_lo, width, tile_idx, tile_off)."""
    return [(lo, w, ti, toff) for (p, lo, w, ti, toff) in Q_PIECES if p == pg]


def emit_body(ctx, tc, outs, ins, n_tiles):
    """ins order: xsT, x0T, x1T, x2T, onehot, Wg1, bg1, Wg2x, EU*5, EV*5, C*5, R, I9
    outs: [res (n_tiles*WIN, 9)]"""
    nc = tc.nc
    A = A_TILE
    (d_xsT, d_x0T, d_x1T, d_x2T, d_oneh, d_Wg1, d_bg1, d_Wg2x,
     d_EU00, d_EU02, d_EU11, d_EU20, d_EU22,
     d_EV00, d_EV02, d_EV11, d_EV20, d_EV22,
     d_C00, d_C02, d_C11, d_C20, d_C22, d_R, d_I9) = ins
    d_EU = {(0, 0): d_EU00, (0, 2): d_EU02, (1, 1): d_EU11, (2, 0): d_EU20, (2, 2): d_EU22}
    d_EV = {(0, 0): d_EV00, (0, 2): d_EV02, (1, 1): d_EV11, (2, 0): d_EV20, (2, 2): d_EV22}
    d_C = {(0, 0): d_C00, (0, 2): d_C02, (1, 1): d_C11, (2, 0): d_C20, (2, 2): d_C22}
    d_res = outs[0]

    SRC = {0: d_x0T, 1: d_x1T, 2: d_x2T}   # feature-major dram per l
    SRC_ROWS = {0: 128, 1: 192, 2: 160}

    # ---------------- persistent weights in SBUF ----------------
    wpool = ctx.enter_context(tc.tile_pool(name="weights", bufs=1))

    def load_w(dram, rows, cols, tag):
        chunks = []
        for (o, c) in _ksplits(rows):
            t = wpool.tile([c, cols], MMDT, tag=f"{tag}_{o}")
            nc.sync.dma_start(t[:], dram[o:o + c, :])
            chunks.append((t, o, c))
        return chunks

    sb_Wg1 = wpool.tile([128, 64], MMDT, tag="Wg1")
    nc.sync.dma_start(sb_Wg1[:], d_Wg1[:])
    sb_bg1 = wpool.tile([64, 1], f32, tag="bg1")
    nc.sync.dma_start(sb_bg1[:], d_bg1[:])
    sb_Wg2x = wpool.tile([65, NQ_PAD], MMDT, tag="Wg2x")
    nc.sync.dma_start(sb_Wg2x[:], d_Wg2x[:])
    sb_I9 = wpool.tile([9, 9], f32, tag="I9")
    nc.sync.dma_start(sb_I9[:], d_I9[:])
    sb_EU, sb_EV, sb_C = {}, {}, {}
    for (pg, ks) in PAIR_GROUPS:
        l1, l2 = pg
        nrow = HID * DIM[l1] * DIM[l2]
        ncol = Q_BLOCK_N[pg]
        sb_EU[pg] = load_w(d_EU[pg], MUL[l1] * DIM[l1], nrow, f"EU{l1}{l2}")
        sb_EV[pg] = load_w(d_EV[pg], MUL[l2] * DIM[l2], nrow, f"EV{l1}{l2}")
        sb_C[pg] = load_w(d_C[pg], nrow, ncol, f"C{l1}{l2}")
    sb_R = load_w(d_R, NQ_PAD, 9, "R")

    # ---------------- pools ----------------
    xin = ctx.enter_context(tc.tile_pool(name="xin", bufs=3))
    oh_pool = ctx.enter_context(tc.tile_pool(name="oneh", bufs=3))
    ps_mm = ctx.enter_context(tc.tile_pool(name="ps_mm", bufs=3, space="PSUM"))
    ps_q = ctx.enter_context(tc.tile_pool(name="ps_q", bufs=2, space="PSUM"))
    ps_sm = ctx.enter_context(tc.tile_pool(name="ps_sm", bufs=1, space="PSUM"))
    sb_s = ctx.enter_context(tc.tile_pool(name="sb_s", bufs=2))
    sb_w = ctx.enter_context(tc.tile_pool(name="sb_w", bufs=2))
    sb_p = ctx.enter_context(tc.tile_pool(name="sb_p", bufs=2))
    sb_out = ctx.enter_context(tc.tile_pool(name="sb_out", bufs=2))

    for t in range(n_tiles):
        a0 = t * A
        # ---- loads (feature-major: [rows, A]) ----
        x_sb = {}
        xs_t = xin.tile([128, A], MMDT, tag="xs")
        nc.sync.dma_start(xs_t[:], d_xsT[:, a0:a0 + A])
        for l in (0, 1, 2):
            rows = SRC_ROWS[l]
            chunks = []
            for (o, c) in _ksplits(rows):
                xt = xin.tile([c, A], MMDT, tag=f"x{l}_{o}")
                nc.sync.dma_start(xt[:], SRC[l][o:o + c, a0:a0 + A])
                chunks.append((xt, o, c))
            x_sb[l] = chunks
        oh_t = oh_pool.tile([128, 4 * WIN], f32, tag="oneh")
        for b in range(4):
            nc.sync.dma_start(oh_t[:, b * WIN:(b + 1) * WIN],
                              d_oneh[a0 + b * 128: a0 + (b + 1) * 128, :])

        # ---- MLP ----
        h1_ps = ps_mm.tile([64, A], f32, tag="mm")
        nc.tensor.matmul(h1_ps[:], _mm_dt(sb_Wg1[:]), _mm_dt(xs_t[:]),
                         start=True, stop=True)
        s_aug = sb_s.tile([65, A], MMDT, tag="s")
        sg = sb_s.tile([64, A], f32, tag="sg")
        nc.scalar.activation(sg[:], h1_ps[:],
                             mybir.ActivationFunctionType.Sigmoid, bias=sb_bg1[:])
        h1b = sb_s.tile([64, A], f32, tag="h1b")
        nc.scalar.activation(h1b[:], h1_ps[:],
                             mybir.ActivationFunctionType.Identity, bias=sb_bg1[:])
        nc.vector.tensor_mul(s_aug[0:64, :], h1b[:], sg[:])
        nc.gpsimd.memset(s_aug[64:65, :].bitcast(f32), 1.0)

        w_sb = []
        for qi, qn in enumerate(Q_TILE_SIZES):
            qo = qi * 128
            wp = ps_mm.tile([128, A], f32, tag="mm")
            nc.tensor.matmul(wp[0:qn, :], _mm_dt(sb_Wg2x[:, qo:qo + qn]),
                             _mm_dt(s_aug[:]), start=True, stop=True)
            wt = sb_w.tile([128, A], f32, tag=f"w{qi}")
            nc.scalar.activation(wt[0:qn, :], wp[0:qn, :],
                                 mybir.ActivationFunctionType.Copy)
            w_sb.append(wt)

        # ---- pair groups: hUe/hVe -> P ----
        P_sb = {}   # pg -> list of (tile, local_row_off, rows)
        for (pg, ks) in PAIR_GROUPS:
            l1, l2 = pg
            nrow = HID * DIM[l1] * DIM[l2]
            plist = []
            for (po, pc) in _ksplits(nrow):
                hu = ps_mm.tile([128, A], f32, tag="mm")
                for ci, (xt, xo, xc) in enumerate(x_sb[l1]):
                    wchunk = next(wt for (wt, wo, wc) in sb_EU[pg] if wo == xo)
                    nc.tensor.matmul(hu[0:pc, :], _mm_dt(wchunk[:, po:po + pc]),
                                     _mm_dt(xt[:]), start=(ci == 0),
                                     stop=(ci == len(x_sb[l1]) - 1))
                hv = ps_mm.tile([128, A], f32, tag="mm")
                for ci, (xt, xo, xc) in enumerate(x_sb[l2]):
                    wchunk = next(wt for (wt, wo, wc) in sb_EV[pg] if wo == xo)
                    nc.tensor.matmul(hv[0:pc, :], _mm_dt(wchunk[:, po:po + pc]),
                                     _mm_dt(xt[:]), start=(ci == 0),
                                     stop=(ci == len(x_sb[l2]) - 1))
                hu_sb = sb_p.tile([128, A], f32, tag="hu")
                nc.scalar.activation(hu_sb[0:pc, :], hu[0:pc, :],
                                     mybir.ActivationFunctionType.Copy)
                pt = sb_p.tile([128, A], MMDT, tag=f"P{l1}{l2}_{po}")
                nc.vector.tensor_mul(pt[0:pc, :], hu_sb[0:pc, :], hv[0:pc, :])
                plist.append((pt, po, pc))
            P_sb[pg] = plist

        # ---- Q accumulation per pair-group M-split; fused evac with wQ ----
        wq_sb = [sb_out.tile([128, A], MMDT, tag=f"wq{i}", name=f"wq{i}")
                 for i in range(N_QT)]
        # zero the pad rows the R matmul will read (legal windows only)
        nc.gpsimd.memset(wq_sb[2][64:128, :].bitcast(f32), 0.0)
        nc.gpsimd.memset(wq_sb[3][64:128, :].bitcast(f32), 0.0)
        nc.gpsimd.memset(wq_sb[4][0:32, :].bitcast(f32), 0.0)
        for (pg, ks) in PAIR_GROUPS:
            for (lo, width, qt, qoff) in _msplits_for_pg(pg):
                q_ps = ps_q.tile([128, A], f32, tag="q", name="q")
                csrcs = sb_C[pg]
                psrcs = P_sb[pg]
                for ci, ((ct, co, cc), (ptile, po, pc)) in enumerate(zip(csrcs, psrcs)):
                    assert co == po and cc == pc
                    nc.tensor.matmul(q_ps[0:width, :], _mm_dt(ct[:, lo:lo + width]),
                                     _mm_dt(ptile[0:pc, :]),
                                     start=(ci == 0), stop=(ci == len(csrcs) - 1))
                nc.vector.tensor_mul(wq_sb[qt][qoff:qoff + width, :],
                                     w_sb[qt][qoff:qoff + width, :],
                                     q_ps[0:width, :])
        ao_ps = ps_sm.tile([9, A], f32, tag="ao")
        for i in range(N_QT):
            qn = Q_TILE_SIZES[i]
            (rt, ro, rc) = sb_R[i]
            assert ro == i * 128 and rc == qn
            nc.tensor.matmul(ao_ps[:], _mm_dt(rt[:]), _mm_dt(wq_sb[i][0:qn, :]),
                             start=(i == 0), stop=(i == N_QT - 1))
        ao_sb = sb_out.tile([9, A], f32, tag="aosb")
        nc.scalar.activation(ao_sb[:], ao_ps[:], mybir.ActivationFunctionType.Copy)

        # ---- segment sum into WIN-graph window ----
        res_ps = ps_sm.tile([WIN, 9], f32, tag="res")
        for b in range(4):
            tr_ps = ps_sm.tile([128, 9], f32, tag="tr")
            nc.tensor.transpose(tr_ps[:], ao_sb[:, b * 128:(b + 1) * 128], sb_I9[:])
            at_sb = sb_out.tile([128, 9], f32, tag="atsb")
            nc.vector.tensor_copy(at_sb[:], tr_ps[:])
            nc.tensor.matmul(res_ps[:], _mm_dt(oh_t[:, b * WIN:(b + 1) * WIN]),
                             _mm_dt(at_sb[:]), start=(b == 0), stop=(b == 3))
        res_sb = sb_out.tile([WIN, 9], f32, tag="ressb")
        nc.vector.tensor_copy(res_sb[:], res_ps[:])
        nc.sync.dma_start(d_res[t * WIN:(t + 1) * WIN, :], res_sb[:])


# ----------------------------------------------------------------------------
# host orchestration
# ----------------------------------------------------------------------------
def prep_core_inputs(inp, C):
    """Shard + transpose activations, build one-hot windows. Returns
    (in_maps, anchors, n_tiles)."""
    xs = np.ascontiguousarray(np.asarray(inp['x_scalar'], np.float32))
    xsph = np.ascontiguousarray(np.asarray(inp['x_spherical'], np.float32))
    batch = np.asarray(inp['batch']).astype(np.int64)
    N = xs.shape[0]
    per = (N + N_CORES - 1) // N_CORES
    n_tiles = (per + A_TILE - 1) // A_TILE
    NP = n_tiles * A_TILE

    weights_map = dict(
        Wg1=C['Wg1'], bg1=C['bg1'], Wg2x=C['Wg2x'], R=C['R'],
        I9=np.eye(9, dtype=np.float32))
    for (pg, ks) in PAIR_GROUPS:
        l1, l2 = pg
        weights_map[f"EU{l1}{l2}"] = C['EU'][pg]
        weights_map[f"EV{l1}{l2}"] = C['EV'][pg]
        weights_map[f"C{l1}{l2}"] = C['Cpg'][pg]

    in_maps, anchors = [], []
    for c in range(N_CORES):
        s, e = c * per, min((c + 1) * per, N)
        n = e - s
        xsT = np.zeros((128, NP), np.float32)
        xsT[:, :n] = xs[s:e].T
        xsphT = np.zeros((480, NP), np.float32)
        xsphT[:, :n] = xsph[s:e].T
        oneh = np.zeros((NP, WIN), np.float32)
        anc = np.zeros(n_tiles, np.int64)
        b = batch[s:e]
        for t in range(n_tiles):
            t0 = t * A_TILE
            if t0 >= n:
                anc[t] = anc[t - 1] if t > 0 else 0
                continue
            t1 = min(t0 + A_TILE, n)
            a = int(b[t0])
            anc[t] = a
            span = int(b[t1 - 1]) - a
            assert span < WIN, f"graph window overflow: span={span}"
            oneh[np.arange(t0, t1), b[t0:t1] - a] = 1.0
        m = dict(weights_map)
        m['xsT'] = xsT
        m['x0T'] = np.ascontiguousarray(xsphT[0:128])
        m['x1T'] = np.ascontiguousarray(xsphT[128:320])
        m['x2T'] = np.ascontiguousarray(xsphT[320:480])
        m['oneh'] = oneh
        in_maps.append(m)
        anchors.append(anc)
    return in_maps, anchors, n_tiles


_PROGRAM_CACHE = {}


def build_program(n_tiles):
    key = (n_tiles, A_TILE, USE_F32R)
    if key in _PROGRAM_CACHE:
        return _PROGRAM_CACHE[key]
    from contextlib import ExitStack
    nc = bacc.Bacc("TRN2", target_bir_lowering=False, debug=False,
                   num_devices=N_CORES)
    NP = n_tiles * A_TILE
    names = ['xsT', 'x0T', 'x1T', 'x2T', 'oneh', 'Wg1', 'bg1', 'Wg2x',
             'EU00', 'EU02', 'EU11', 'EU20', 'EU22',
             'EV00', 'EV02', 'EV11', 'EV20', 'EV22',
             'C00', 'C02', 'C11', 'C20', 'C22', 'R', 'I9']
    shapes = dict(
        xsT=(128, NP), x0T=(128, NP), x1T=(192, NP), x2T=(160, NP),
        oneh=(NP, WIN), Wg1=(128, 64), bg1=(64, 1), Wg2x=(65, NQ_PAD), R=(NQ_PAD, 9),
        I9=(9, 9))
    for (pg, ks) in PAIR_GROUPS:
        l1, l2 = pg
        nrow = HID * DIM[l1] * DIM[l2]
        ncol = Q_BLOCK_N[pg]
        shapes[f"EU{l1}{l2}"] = (MUL[l1] * DIM[l1], nrow)
        shapes[f"EV{l1}{l2}"] = (MUL[l2] * DIM[l2], nrow)
        shapes[f"C{l1}{l2}"] = (nrow, ncol)
    ins = [nc.dram_tensor(nm, list(shapes[nm]),
                          f32 if nm in ('bg1', 'oneh', 'I9') else MMDT,
                          kind="ExternalInput").ap()
           for nm in names]
    out = nc.dram_tensor("res", [n_tiles * WIN, 9], f32, kind="ExternalOutput").ap()

    with tile.TileContext(nc) as tc:
        with ExitStack() as ctx:
            emit_body(ctx, tc, [out], ins, n_tiles)
    nc.compile()
    _PROGRAM_CACHE[key] = nc
    return nc


def kernel(**inputs):
    from concourse.bass_utils import run_bass_kernel_spmd
    C = build_constants(inputs)
    in_maps, anchors, n_tiles = prep_core_inputs(inputs, C)
    nc = build_program(n_tiles)
    results = run_bass_kernel_spmd(nc, in_maps, core_ids=list(range(N_CORES))).results

    G = int(inputs['num_graphs'])
    res_sph = np.zeros((G, 9), np.float64)
    for c in range(N_CORES):
        r = results[c]['res'].reshape(n_tiles, WIN, 9)
        for t in range(n_tiles):
            a = int(anchors[c][t])
            hi = min(a + WIN, G)
            res_sph[a:hi] += r[t][:hi - a]
    out9 = res_sph.astype(np.float32) @ C['M_cart']
    return out9.reshape(G, 3, 3)
